# revision 25
# baseline (speedup 1.0000x reference)
"""Trainium2 Bass kernel for nn_NodeProcessor (GNN message passing).

Computation (per reference):
    agg = segment_sum(edge_attr, col=edge_index[1], N)      # [N, 64]
    h = relu(concat([x, agg]) @ W0 + b0)
    h = relu(h @ W1 + b1)
    h = h @ W2 + b2
    out = layernorm(h) * ln_g + ln_b + x

Distribution: destination-sharded edges, no collectives. Nodes are
degree-sorted globally and dealt round-robin across the 8 cores so every
core sees the IDENTICAL per-bucket degree profile (one SPMD program).
Each core owns 12800 nodes in 100 buckets of 128. Bucket order: four
mid-degree pairs first (fills the MLP pipeline immediately), then
heaviest-to-lightest so light buckets drain the tail. Bucket pairs share
the 128 SBUF partitions feature-major: bucket A's 64 edge features on
partitions 0:64, bucket B's on 64:128, nodes along the free dim; each
pair is padded to its max degree (~2% pad, zero-filled).

Per-core device kernel (identical SPMD program, compiled per degree
profile -- the profile is index metadata, all value compute is on-device):
  Scatter: segment-sum = (d-1) DVE tensor_tensor adds per equal-degree
  pair chunk (bf16 packed operands -> DVE 2x mode). No one-hots, no
  per-128-edge matmuls (the v0 baseline's 2000-matmul scatter is gone).
  The aggregate lands DIRECTLY feature-major, so layer 0 consumes it via
  two [W0a;0]/[0;W0a] matmuls with strided PSUM output -- zero
  transposes, zero copies between scatter and MLP.
  MLP: bf16 weights/activations, 512-column moving matmuls (1 cycle/row
  vs 4 for fp32). LayerNorm: PE-transposes h2 node-major into a PSUM
  ring 8 mega-tiles deep (2 banks; depth hides the stats/apply chain
  latency), bn_stats/bn_aggr on DVE, rstd via fast approximate
  reciprocal, mean*rstd on the otherwise idle GpSimd, apply as
  scalar-engine scale+bias straight out of PSUM, ln_g-mult + residual on
  GpSimd. ln_b is folded into the residual x on the host; output ships
  bf16 (gate is 2e-2; measured rel err 7e-3).

Measured on trn2 (8 cores): 159.8us vs 528.8us for the one-hot matmul
baseline (3.3x). Engine busy: DVE ~97us, Act ~80us, PE ~68us, DMA
~100us active (26MB/core moved at the 16-engine descriptor rate).
"""

import numpy as np
import ml_dtypes
from contextlib import ExitStack

from concourse import bacc, mybir
from concourse.tile import TileContext
from concourse.bass_utils import run_bass_kernel_spmd

BF16 = ml_dtypes.bfloat16

N_NODES = 100000
N_EDGES = 1200000
D = 64          # d_node == d_edge
H = 128         # hidden
EPS = 1e-5
NCORES = 8
P = 128
NPC = 12800     # nodes per core (padded)
NB = 100        # buckets of 128 nodes per core
NPAIR = NB // 2
MB = 512        # MLP mega-tile (nodes)
MT = NPC // MB  # 25 mega-tiles per core
CHCAP = 128     # chunk cap: npairs*d <= CHCAP (32KB/partition bf16)
POOL_COLS = 0      # scatter offload to GpSimd disabled (stalls pipeline)
POOL_DRANGE = (5, 10)

_prog_cache: dict[tuple, object] = {}
last_results = None


def _build_program(chunks):
    """chunks: tuple of (d, p0, np_, off) with off in d-slot units."""
    Fp = chunks[-1][3] + chunks[-1][0] * chunks[-1][2]  # pair slots
    nc = bacc.Bacc("TRN2", target_bir_lowering=False, debug=False,
                   num_devices=NCORES)
    f32 = mybir.dt.float32
    bf16 = mybir.dt.bfloat16
    AF = mybir.ActivationFunctionType
    ALU = mybir.AluOpType

    attr_d = nc.dram_tensor("attr", [P, Fp * P], bf16, kind="ExternalInput").ap()
    xT_d = nc.dram_tensor("xT", [D, NPC], bf16, kind="ExternalInput").ap()
    xb_d = nc.dram_tensor("xb", [P, NB * D], bf16, kind="ExternalInput").ap()
    wb_d = nc.dram_tensor("wb", [P, 904], bf16, kind="ExternalInput").ap()
    cf_d = nc.dram_tensor("cf", [P, 4], f32, kind="ExternalInput").ap()
    out_d = nc.dram_tensor("out", [P, NB * D], bf16, kind="ExternalOutput").ap()

    with TileContext(nc) as tc, ExitStack() as ctx:
        const = ctx.enter_context(tc.tile_pool(name="const", bufs=1))
        chp = ctx.enter_context(tc.tile_pool(name="chp", bufs=3))
        xtp = ctx.enter_context(tc.tile_pool(name="xtp", bufs=3))
        xbp = ctx.enter_context(tc.tile_pool(name="xbp", bufs=8))
        h0sp = ctx.enter_context(tc.tile_pool(name="h0sp", bufs=3))
        h1sp = ctx.enter_context(tc.tile_pool(name="h1sp", bufs=3))
        h2sp = ctx.enter_context(tc.tile_pool(name="h2sp", bufs=4))
        zp = ctx.enter_context(tc.tile_pool(name="zp", bufs=8))
        y1p = ctx.enter_context(tc.tile_pool(name="y1p", bufs=8))
        outp = ctx.enter_context(tc.tile_pool(name="outp", bufs=8))
        stp = ctx.enter_context(tc.tile_pool(name="stp", bufs=8))
        ps_h0 = ctx.enter_context(tc.tile_pool(name="ps_h0", bufs=2, space="PSUM"))
        ps_h1 = ctx.enter_context(tc.tile_pool(name="ps_h1", bufs=2, space="PSUM"))
        ps_h2 = ctx.enter_context(tc.tile_pool(name="ps_h2", bufs=2, space="PSUM"))
        ps_nm = ctx.enter_context(tc.tile_pool(name="ps_nm", bufs=1, space="PSUM"))

        wb = const.tile([P, 904], bf16, tag="wb")
        nc.sync.dma_start(out=wb[:], in_=wb_d[:])
        cf = const.tile([P, 4], f32, tag="cf")
        nc.sync.dma_start(out=cf[:], in_=cf_d[:])
        w0x = wb[0:D, 0:H]
        w0a_t = wb[:, 128:256]      # [W0a; 0]
        w0a_b = wb[:, 256:384]      # [0; W0a]
        w1 = wb[:, 384:512]
        w2 = wb[:, 512:576]
        i64 = wb[0:D, 576:640]
        gt = wb[:, 640:896]         # ln_g tiled 4x, all partitions
        b0c = cf[:, 0:1]
        b1c = cf[:, 1:2]
        b2c = cf[0:D, 2:3]
        epsc = cf[:, 3:4]

        # persistent feature-major aggregate: pair p -> cols [p*128,(p+1)*128)
        acc = const.tile([P, NPAIR * P], bf16, tag="acc")
        # rotating node-major h2 (8 mega-tiles deep, 2 PSUM banks)
        nm = ps_nm.tile([P, 8 * 256], bf16, tag="nm")

        # ---- scatter: one chunk of np_ same-degree pairs ----
        def emit_chunk(d, p0, np_, off, eng):
            ch = chp.tile([P, np_ * d * P], bf16, tag="ch", name=f"ch{p0}")
            nc.sync.dma_start(out=ch[:], in_=attr_d[:, off * P:(off + np_ * d) * P])
            accv = acc[:, p0 * P:(p0 + np_) * P].rearrange(
                "p (b n) -> p b n", b=np_)
            if d == 1:
                nc.scalar.activation(out=acc[:, p0 * P:(p0 + np_) * P],
                                     in_=ch[:], func=AF.Copy)
                return
            chv = ch[:].rearrange("p (b j n) -> p b j n", b=np_, j=d)
            eng.tensor_tensor(out=accv, in0=chv[:, :, 0, :],
                              in1=chv[:, :, 1, :], op=ALU.add)
            for j in range(2, d):
                eng.tensor_tensor(out=accv, in0=accv,
                                  in1=chv[:, :, j, :], op=ALU.add)

        # ---- MLP stages over 512-node mega-tiles ----
        xts, xbs = {}, {}
        h0ss, h1ss, h2ss, stats, zs = {}, {}, {}, {}, {}

        def s0(t):
            xt = xtp.tile([D, MB], bf16, tag="xt", name=f"xt{t}")
            nc.sync.dma_start(out=xt[:], in_=xT_d[:, t * MB:(t + 1) * MB])
            xts[t] = xt

        def s1(t):
            h0 = ps_h0.tile([H, MB], f32, tag="h0", name=f"h0_{t}")
            nc.tensor.matmul(out=h0[:], lhsT=w0x, rhs=xts.pop(t)[:],
                             start=True, stop=False)
            accs = acc[:, 2 * t * P:(2 * t + 2) * P]
            h0v = h0[:].rearrange("p (u v) -> p u v", u=4)
            nc.tensor.matmul(out=h0v[:, 0::2, :], lhsT=w0a_t, rhs=accs,
                             start=False, stop=False, skip_group_check=True)
            nc.tensor.matmul(out=h0v[:, 1::2, :], lhsT=w0a_b, rhs=accs,
                             start=False, stop=True, skip_group_check=True)
            return h0

        def s2(t, h0):
            h0s = h0sp.tile([H, MB], bf16, tag="h0s", name=f"h0s{t}")
            nc.scalar.activation(out=h0s[:], in_=h0[:], func=AF.Relu, bias=b0c)
            h0ss[t] = h0s

        def s3(t):
            h1 = ps_h1.tile([H, MB], f32, tag="h1", name=f"h1_{t}")
            nc.tensor.matmul(out=h1[:], lhsT=w1, rhs=h0ss.pop(t)[:],
                             start=True, stop=True)
            return h1

        def s4(t, h1):
            h1s = h1sp.tile([H, MB], bf16, tag="h1s", name=f"h1s{t}")
            nc.scalar.activation(out=h1s[:], in_=h1[:], func=AF.Relu, bias=b1c)
            h1ss[t] = h1s

        def s5(t):
            h2 = ps_h2.tile([D, MB], f32, tag="h2", name=f"h2_{t}")
            nc.tensor.matmul(out=h2[:], lhsT=w2, rhs=h1ss.pop(t)[:],
                             start=True, stop=True)
            return h2

        def s6(t, h2):
            h2s = h2sp.tile([D, MB], bf16, tag="h2s", name=f"h2s{t}")
            nc.scalar.activation(out=h2s[:], in_=h2[:], func=AF.Identity,
                                 bias=b2c)
            h2ss[t] = h2s

        def s7(t):
            o = (t % 8) * 256
            h2s = h2ss.pop(t)
            for k in range(4):
                nc.tensor.transpose(out=nm[:, o + k * D:o + (k + 1) * D],
                                    in_=h2s[:, k * P:(k + 1) * P],
                                    identity=i64)

        def s8(t):
            o = (t % 8) * 256
            st = stp.tile([P, 44], f32, tag="st", name=f"st{t}")
            stv = st[:, 0:24]
            mv = st[:, 24:32]
            stdc = st[:, 32:36]
            rstd = st[:, 36:40]
            nb_ = st[:, 40:44]
            for g in range(4):
                nc.vector.bn_stats(out=stv[:, 6 * g:6 * g + 6],
                                   in_=nm[:, o + g * D:o + (g + 1) * D])
                nc.vector.bn_aggr(out=mv[:, 2 * g:2 * g + 2],
                                  in_=stv[:, 6 * g:6 * g + 6])
            mvv = mv.rearrange("p (g w) -> p w g", w=2)
            nc.scalar.activation(out=stdc, in_=mvv[:, 1, :], func=AF.Sqrt,
                                 bias=epsc)
            nc.vector.reciprocal_approx_fast(out=rstd, in_=stdc)
            nc.gpsimd.tensor_tensor(out=nb_, in0=mvv[:, 0, :], in1=rstd,
                                    op=ALU.mult)
            nc.gpsimd.tensor_scalar_mul(out=nb_, in0=nb_, scalar1=-1.0)
            stats[t] = st

        def s9(t):
            o = (t % 8) * 256
            st = stats.pop(t)
            z = zp.tile([P, 256], bf16, tag="z", name=f"z{t}")
            for g in range(4):
                nc.scalar.activation(out=z[:, g * D:(g + 1) * D],
                                     in_=nm[:, o + g * D:o + (g + 1) * D],
                                     func=AF.Identity,
                                     scale=st[:, 36 + g:37 + g],
                                     bias=st[:, 40 + g:41 + g])
            zs[t] = z
            xb = xbp.tile([P, 256], bf16, tag="xb", name=f"xb{t}")
            nc.sync.dma_start(out=xb[:], in_=xb_d[:, t * 256:(t + 1) * 256])
            xbs[t] = xb

        def s10(t):
            y1 = y1p.tile([P, 256], bf16, tag="y1", name=f"y1_{t}")
            nc.gpsimd.tensor_tensor(out=y1[:], in0=zs.pop(t)[:], in1=gt,
                                    op=ALU.mult)
            yo = outp.tile([P, 256], bf16, tag="yo", name=f"yo{t}")
            nc.gpsimd.tensor_tensor(out=yo[:], in0=y1[:], in1=xbs.pop(t)[:],
                                    op=ALU.add)
            nc.sync.dma_start(out=out_d[:, t * 256:(t + 1) * 256], in_=yo[:])

        vals = {}
        psfn = {1: s1, 3: s3, 5: s5}
        csfn = {2: s2, 4: s4, 6: s6}
        sfn = {0: s0, 7: s7, 8: s8, 9: s9, 10: s10}

        def run_stage(s, t):
            if not (0 <= t < MT):
                return
            if s in psfn:
                vals[(s + 1, t)] = psfn[s](t)
            elif s in csfn:
                csfn[s](t, vals.pop((s, t)))
            else:
                sfn[s](t)

        NS = 11
        # route mid-degree chunks to gpsimd until POOL_COLS columns
        engs = []
        pc = 0
        for (d, p0, np_, off) in chunks:
            cols = (d - 1) * np_ * P
            if POOL_DRANGE[0] <= d <= POOL_DRANGE[1] and pc < POOL_COLS:
                engs.append(nc.gpsimd)
                pc += cols
            else:
                engs.append(nc.vector)
        ci = 0
        while ci < len(chunks) and chunks[ci][1] < 6:
            emit_chunk(*chunks[ci], engs[ci])
            ci += 1
        for q in range(MT + NS):
            for s in range(NS):
                run_stage(s, q - s)
            need_p = 2 * (q + 4)
            while ci < len(chunks) and chunks[ci][1] < need_p:
                emit_chunk(*chunks[ci], engs[ci])
                ci += 1

    nc.compile()
    return nc


def _host_plan(col):
    """Degree-sort nodes, deal across cores, order buckets desc, pair."""
    NPAD = NCORES * NPC
    deg = np.zeros(NPAD, np.int64)
    deg[:N_NODES] = np.bincount(col, minlength=N_NODES)
    order = np.argsort(deg, kind="stable")          # ascending degree
    dsort = deg[order]
    d_blk = dsort.reshape(NB, NCORES * P).max(axis=1)
    d_blk = np.maximum(d_blk, 1).astype(np.int64)
    bo = np.argsort(-d_blk, kind="stable")          # desc degree
    d_new = d_blk[bo]
    dp0 = np.maximum(d_new[0::2], d_new[1::2])
    # start with 4 mid-degree pairs so the MLP pipeline fills immediately,
    # then heaviest-to-lightest (light pairs drain the tail fast)
    mid = NPAIR // 2
    porder = np.concatenate([np.arange(mid, mid + 4),
                             np.arange(0, mid),
                             np.arange(mid + 4, NPAIR)])
    d_pair = dp0[porder]
    bo = bo.reshape(NPAIR, 2)[porder].reshape(-1)
    inv_bo = np.empty(NB, np.int64)
    inv_bo[bo] = np.arange(NB)
    prefp = np.zeros(NPAIR, np.int64)
    np.cumsum(d_pair[:-1], out=prefp[1:])
    chunks = []
    p = 0
    while p < NPAIR:
        d = int(d_pair[p])
        e = p
        while e < NPAIR and d_pair[e] == d:
            e += 1
        cap = max(1, CHCAP // d)
        while p < e:
            np_ = min(cap, e - p)
            chunks.append((d, p, np_, int(prefp[p])))
            p += np_
    return order, dsort, bo, inv_bo, d_pair, prefp, tuple(chunks)


def _host_pack(col, edge_attr, order, dsort, inv_bo, prefp, Fp):
    E = col.shape[0]
    NPAD = NCORES * NPC
    pos = np.empty(NPAD, np.int64)
    pos[order] = np.arange(NPAD)
    pe = pos[col]                                    # sorted-pos of each dest
    eorder = np.argsort(pe, kind="stable")
    ps = pe[eorder]
    starts = np.zeros(NPAD, np.int64)
    np.cumsum(dsort[:-1], out=starts[1:])
    j = np.arange(E, dtype=np.int64) - starts[ps]
    c = ps % NCORES
    r = ps // NCORES
    b_old = r // P
    lane = r % P
    k = inv_bo[b_old]                                # new bucket index
    pair = k // 2
    half = k % 2
    COLS = Fp * P
    colp = (prefp[pair] + j) * P + lane
    rows = c * COLS + colp
    A = np.zeros((NCORES * COLS, P), BF16)
    av = np.asarray(edge_attr, np.float32)[eorder].astype(BF16)
    m0 = half == 0
    A[rows[m0], 0:D] = av[m0]
    A[rows[~m0], D:P] = av[~m0]
    return np.ascontiguousarray(
        A.reshape(NCORES, COLS, P).transpose(0, 2, 1))


def _host_x(x, ln_b, order, bo):
    NPAD = NCORES * NPC
    xpad = np.zeros((NPAD, D), np.float32)
    xpad[:N_NODES] = np.asarray(x, np.float32)
    # node at (core c, new bucket k, lane) = order[(bo[k]*128+lane)*8 + c]
    r_old = (bo[:, None] * P + np.arange(P)[None, :]).reshape(-1)   # [NPC]
    idx = order[r_old[:, None] * NCORES + np.arange(NCORES)[None, :]]
    xTs, xbs = [], []
    bln = np.asarray(ln_b, np.float32)[None, :]
    for cc in range(NCORES):
        xp = xpad[idx[:, cc]]                        # [NPC, 64]
        xTs.append(np.ascontiguousarray(xp.T).astype(BF16))
        xb = (xp + bln).reshape(NB, P, D).transpose(1, 0, 2).reshape(P, NB * D)
        xbs.append(np.ascontiguousarray(xb).astype(BF16))
    return idx, xTs, xbs


def _host_consts(W0, b0, W1, b1, W2, b2, ln_g):
    wb = np.zeros((P, 904), np.float32)
    W0 = np.asarray(W0, np.float32)
    wb[0:D, 0:H] = W0[0:D]                           # w0x
    wb[0:D, H:2 * H] = W0[D:2 * D]                   # [W0a; 0]
    wb[D:P, 2 * H:3 * H] = W0[D:2 * D]               # [0; W0a]
    wb[:, 384:512] = np.asarray(W1, np.float32)
    wb[:, 512:576] = np.asarray(W2, np.float32)
    wb[0:D, 576:640] = np.eye(D, dtype=np.float32)
    wb[:, 640:896] = np.broadcast_to(
        np.tile(np.asarray(ln_g, np.float32), 4), (P, 256))
    cf = np.zeros((P, 4), np.float32)
    cf[:, 0] = np.asarray(b0, np.float32)
    cf[:, 1] = np.asarray(b1, np.float32)
    cf[0:D, 2] = np.asarray(b2, np.float32)
    cf[:, 3] = EPS
    return wb.astype(BF16), cf


def kernel(x, edge_index, edge_attr, W0, b0, W1, b1, W2, b2, ln_g, ln_b):
    global last_results
    col = np.asarray(edge_index[1]).astype(np.int64)
    order, dsort, bo, inv_bo, d_pair, prefp, chunks = _host_plan(col)
    Fp = int(prefp[-1] + d_pair[-1])

    if chunks not in _prog_cache:
        _prog_cache[chunks] = _build_program(chunks)
    nc = _prog_cache[chunks]

    A = _host_pack(col, edge_attr, order, dsort, inv_bo, prefp, Fp)
    idx, xTs, xbs = _host_x(x, ln_b, order, bo)
    wb, cf = _host_consts(W0, b0, W1, b1, W2, b2, ln_g)

    in_maps = []
    for c in range(NCORES):
        in_maps.append({"attr": A[c], "xT": xTs[c], "xb": xbs[c],
                        "wb": wb, "cf": cf})

    res = run_bass_kernel_spmd(nc, in_maps, core_ids=list(range(NCORES)))
    last_results = res

    out = np.zeros((NCORES * NPC, D), np.float32)
    for c in range(NCORES):
        osw = res.results[c]["out"].astype(np.float32)   # [128, NB*64]
        o3 = osw.reshape(P, NB, D).transpose(1, 0, 2).reshape(NPC, D)
        out[idx[:, c]] = o3
    return np.ascontiguousarray(out[:N_NODES])


# revision 27
# speedup vs baseline: 1.0193x; 1.0193x over previous
"""Trainium2 Bass kernel for nn_NodeProcessor (GNN message passing).

Computation (per reference):
    agg = segment_sum(edge_attr, col=edge_index[1], N)      # [N, 64]
    h = relu(concat([x, agg]) @ W0 + b0)
    h = relu(h @ W1 + b1)
    h = h @ W2 + b2
    out = layernorm(h) * ln_g + ln_b + x

Distribution: destination-sharded edges, no collectives. Nodes are
degree-sorted globally and dealt round-robin across the 8 cores so every
core sees the IDENTICAL per-bucket degree profile (one SPMD program).
Each core owns 12800 nodes in 100 buckets of 128. Bucket order: four
mid-degree pairs first (fills the MLP pipeline immediately), then
heaviest-to-lightest so light buckets drain the tail. Bucket pairs share
the 128 SBUF partitions feature-major: bucket A's 64 edge features on
partitions 0:64, bucket B's on 64:128, nodes along the free dim; each
pair is padded to its max degree (~2% pad, zero-filled).

Per-core device kernel (identical SPMD program, compiled per degree
profile -- the profile is index metadata, all value compute is on-device):
  Scatter: segment-sum = (d-1) DVE tensor_tensor adds per equal-degree
  pair chunk (bf16 packed operands -> DVE 2x mode). No one-hots, no
  per-128-edge matmuls (the v0 baseline's 2000-matmul scatter is gone).
  The aggregate lands DIRECTLY feature-major, so layer 0 consumes it via
  two [W0a;0]/[0;W0a] matmuls with strided PSUM output -- zero
  transposes, zero copies between scatter and MLP.
  MLP: bf16 weights/activations, 512-column moving matmuls (1 cycle/row
  vs 4 for fp32). LayerNorm: PE-transposes h2 node-major into a PSUM
  ring 8 mega-tiles deep (2 banks; depth hides the stats/apply chain
  latency), bn_stats/bn_aggr on DVE, rstd via fast approximate
  reciprocal, mean*rstd on the otherwise idle GpSimd, apply as
  scalar-engine scale+bias straight out of PSUM, ln_g-mult + residual on
  GpSimd. ln_b is folded into the residual x on the host; output ships
  bf16 (gate is 2e-2; measured rel err 7e-3).

Measured on trn2 (8 cores): 159.8us vs 528.8us for the one-hot matmul
baseline (3.3x). Engine busy: DVE ~97us, Act ~80us, PE ~68us, DMA
~100us active (26MB/core moved at the 16-engine descriptor rate).
"""

import numpy as np
import ml_dtypes
from contextlib import ExitStack

from concourse import bacc, mybir
from concourse.tile import TileContext
from concourse.bass_utils import run_bass_kernel_spmd

BF16 = ml_dtypes.bfloat16

N_NODES = 100000
N_EDGES = 1200000
D = 64          # d_node == d_edge
H = 128         # hidden
EPS = 1e-5
NCORES = 8
P = 128
NPC = 12800     # nodes per core (padded)
NB = 100        # buckets of 128 nodes per core
NPAIR = NB // 2
MB = 512        # MLP mega-tile (nodes)
MT = NPC // MB  # 25 mega-tiles per core
CHCAP = 160     # chunk cap: npairs*d <= CHCAP (40KB/partition bf16)
POOL_COLS = 0      # scatter offload to GpSimd disabled (stalls pipeline)
POOL_DRANGE = (5, 10)

_prog_cache: dict[tuple, object] = {}
last_results = None


def _build_program(chunks):
    """chunks: tuple of (d, p0, np_, off) with off in d-slot units."""
    Fp = chunks[-1][3] + chunks[-1][0] * chunks[-1][2]  # pair slots
    nc = bacc.Bacc("TRN2", target_bir_lowering=False, debug=False,
                   num_devices=NCORES)
    f32 = mybir.dt.float32
    bf16 = mybir.dt.bfloat16
    AF = mybir.ActivationFunctionType
    ALU = mybir.AluOpType

    attr_d = nc.dram_tensor("attr", [P, Fp * P], bf16, kind="ExternalInput").ap()
    xT_d = nc.dram_tensor("xT", [D, NPC], bf16, kind="ExternalInput").ap()
    xb_d = nc.dram_tensor("xb", [P, NB * D], bf16, kind="ExternalInput").ap()
    wb_d = nc.dram_tensor("wb", [P, 904], bf16, kind="ExternalInput").ap()
    cf_d = nc.dram_tensor("cf", [P, 4], f32, kind="ExternalInput").ap()
    out_d = nc.dram_tensor("out", [P, NB * D], bf16, kind="ExternalOutput").ap()

    with TileContext(nc) as tc, ExitStack() as ctx:
        const = ctx.enter_context(tc.tile_pool(name="const", bufs=1))
        chp = ctx.enter_context(tc.tile_pool(name="chp", bufs=3))
        xtp = ctx.enter_context(tc.tile_pool(name="xtp", bufs=3))
        xbp = ctx.enter_context(tc.tile_pool(name="xbp", bufs=3))
        h0sp = ctx.enter_context(tc.tile_pool(name="h0sp", bufs=3))
        h1sp = ctx.enter_context(tc.tile_pool(name="h1sp", bufs=3))
        h2sp = ctx.enter_context(tc.tile_pool(name="h2sp", bufs=3))
        zp = ctx.enter_context(tc.tile_pool(name="zp", bufs=3))
        y1p = ctx.enter_context(tc.tile_pool(name="y1p", bufs=3))
        outp = ctx.enter_context(tc.tile_pool(name="outp", bufs=4))
        stp = ctx.enter_context(tc.tile_pool(name="stp", bufs=3))
        ps_h0 = ctx.enter_context(tc.tile_pool(name="ps_h0", bufs=2, space="PSUM"))
        ps_h1 = ctx.enter_context(tc.tile_pool(name="ps_h1", bufs=2, space="PSUM"))
        ps_h2 = ctx.enter_context(tc.tile_pool(name="ps_h2", bufs=2, space="PSUM"))
        ps_nm = ctx.enter_context(tc.tile_pool(name="ps_nm", bufs=1, space="PSUM"))

        wb = const.tile([P, 904], bf16, tag="wb")
        nc.sync.dma_start(out=wb[:], in_=wb_d[:])
        cf = const.tile([P, 4], f32, tag="cf")
        nc.sync.dma_start(out=cf[:], in_=cf_d[:])
        w0x = wb[0:D, 0:H]
        w0a_t = wb[:, 128:256]      # [W0a; 0]
        w0a_b = wb[:, 256:384]      # [0; W0a]
        w1 = wb[:, 384:512]
        w2 = wb[:, 512:576]
        i64 = wb[0:D, 576:640]
        gt = wb[:, 640:896]         # ln_g tiled 4x, all partitions
        b0c = cf[:, 0:1]
        b1c = cf[:, 1:2]
        b2c = cf[0:D, 2:3]
        epsc = cf[:, 3:4]

        # persistent feature-major aggregate: pair p -> cols [p*128,(p+1)*128)
        acc = const.tile([P, NPAIR * P], bf16, tag="acc")
        # rotating node-major h2 (8 mega-tiles deep, 2 PSUM banks)
        nm = ps_nm.tile([P, 8 * 256], bf16, tag="nm")

        # ---- scatter: one chunk of np_ same-degree pairs ----
        def emit_chunk(d, p0, np_, off, eng):
            ch = chp.tile([P, np_ * d * P], bf16, tag="ch", name=f"ch{p0}")
            nc.sync.dma_start(out=ch[:], in_=attr_d[:, off * P:(off + np_ * d) * P])
            accv = acc[:, p0 * P:(p0 + np_) * P].rearrange(
                "p (b n) -> p b n", b=np_)
            if d == 1:
                nc.scalar.activation(out=acc[:, p0 * P:(p0 + np_) * P],
                                     in_=ch[:], func=AF.Copy)
                return
            chv = ch[:].rearrange("p (b j n) -> p b j n", b=np_, j=d)
            eng.tensor_tensor(out=accv, in0=chv[:, :, 0, :],
                              in1=chv[:, :, 1, :], op=ALU.add)
            for j in range(2, d):
                eng.tensor_tensor(out=accv, in0=accv,
                                  in1=chv[:, :, j, :], op=ALU.add)

        # ---- MLP stages over 512-node mega-tiles ----
        xts, xbs = {}, {}
        h0ss, h1ss, h2ss, stats, zs = {}, {}, {}, {}, {}

        def s0(t):
            xt = xtp.tile([D, MB], bf16, tag="xt", name=f"xt{t}")
            nc.sync.dma_start(out=xt[:], in_=xT_d[:, t * MB:(t + 1) * MB])
            xts[t] = xt

        def s1(t):
            h0 = ps_h0.tile([H, MB], f32, tag="h0", name=f"h0_{t}")
            nc.tensor.matmul(out=h0[:], lhsT=w0x, rhs=xts.pop(t)[:],
                             start=True, stop=False)
            accs = acc[:, 2 * t * P:(2 * t + 2) * P]
            h0v = h0[:].rearrange("p (u v) -> p u v", u=4)
            nc.tensor.matmul(out=h0v[:, 0::2, :], lhsT=w0a_t, rhs=accs,
                             start=False, stop=False, skip_group_check=True)
            nc.tensor.matmul(out=h0v[:, 1::2, :], lhsT=w0a_b, rhs=accs,
                             start=False, stop=True, skip_group_check=True)
            return h0

        def s2(t, h0):
            h0s = h0sp.tile([H, MB], bf16, tag="h0s", name=f"h0s{t}")
            nc.scalar.activation(out=h0s[:], in_=h0[:], func=AF.Relu, bias=b0c)
            h0ss[t] = h0s

        def s3(t):
            h1 = ps_h1.tile([H, MB], f32, tag="h1", name=f"h1_{t}")
            nc.tensor.matmul(out=h1[:], lhsT=w1, rhs=h0ss.pop(t)[:],
                             start=True, stop=True)
            return h1

        def s4(t, h1):
            h1s = h1sp.tile([H, MB], bf16, tag="h1s", name=f"h1s{t}")
            nc.scalar.activation(out=h1s[:], in_=h1[:], func=AF.Relu, bias=b1c)
            h1ss[t] = h1s

        def s5(t):
            h2 = ps_h2.tile([D, MB], f32, tag="h2", name=f"h2_{t}")
            nc.tensor.matmul(out=h2[:], lhsT=w2, rhs=h1ss.pop(t)[:],
                             start=True, stop=True)
            return h2

        def s6(t, h2):
            h2s = h2sp.tile([D, MB], bf16, tag="h2s", name=f"h2s{t}")
            nc.scalar.activation(out=h2s[:], in_=h2[:], func=AF.Identity,
                                 bias=b2c)
            h2ss[t] = h2s

        def s7(t):
            o = (t % 8) * 256
            h2s = h2ss.pop(t)
            for k in range(4):
                nc.tensor.transpose(out=nm[:, o + k * D:o + (k + 1) * D],
                                    in_=h2s[:, k * P:(k + 1) * P],
                                    identity=i64)

        def s8(t):
            o = (t % 8) * 256
            st = stp.tile([P, 44], f32, tag="st", name=f"st{t}")
            stv = st[:, 0:24]
            mv = st[:, 24:32]
            stdc = st[:, 32:36]
            rstd = st[:, 36:40]
            nb_ = st[:, 40:44]
            for g in range(4):
                nc.vector.bn_stats(out=stv[:, 6 * g:6 * g + 6],
                                   in_=nm[:, o + g * D:o + (g + 1) * D])
                nc.vector.bn_aggr(out=mv[:, 2 * g:2 * g + 2],
                                  in_=stv[:, 6 * g:6 * g + 6])
            mvv = mv.rearrange("p (g w) -> p w g", w=2)
            nc.scalar.activation(out=stdc, in_=mvv[:, 1, :], func=AF.Sqrt,
                                 bias=epsc)
            nc.vector.reciprocal_approx_fast(out=rstd, in_=stdc)
            nc.gpsimd.tensor_tensor(out=nb_, in0=mvv[:, 0, :], in1=rstd,
                                    op=ALU.mult)
            nc.gpsimd.tensor_scalar_mul(out=nb_, in0=nb_, scalar1=-1.0)
            stats[t] = st

        def s9(t):
            o = (t % 8) * 256
            st = stats.pop(t)
            z = zp.tile([P, 256], bf16, tag="z", name=f"z{t}")
            for g in range(4):
                nc.scalar.activation(out=z[:, g * D:(g + 1) * D],
                                     in_=nm[:, o + g * D:o + (g + 1) * D],
                                     func=AF.Identity,
                                     scale=st[:, 36 + g:37 + g],
                                     bias=st[:, 40 + g:41 + g])
            zs[t] = z
            xb = xbp.tile([P, 256], bf16, tag="xb", name=f"xb{t}")
            nc.sync.dma_start(out=xb[:], in_=xb_d[:, t * 256:(t + 1) * 256])
            xbs[t] = xb

        def s10(t):
            y1 = y1p.tile([P, 256], bf16, tag="y1", name=f"y1_{t}")
            nc.gpsimd.tensor_tensor(out=y1[:], in0=zs.pop(t)[:], in1=gt,
                                    op=ALU.mult)
            yo = outp.tile([P, 256], bf16, tag="yo", name=f"yo{t}")
            nc.gpsimd.tensor_tensor(out=yo[:], in0=y1[:], in1=xbs.pop(t)[:],
                                    op=ALU.add)
            nc.sync.dma_start(out=out_d[:, t * 256:(t + 1) * 256], in_=yo[:])

        vals = {}
        psfn = {1: s1, 3: s3, 5: s5}
        csfn = {2: s2, 4: s4, 6: s6}
        sfn = {0: s0, 7: s7, 8: s8, 9: s9, 10: s10}

        def run_stage(s, t):
            if not (0 <= t < MT):
                return
            if s in psfn:
                vals[(s + 1, t)] = psfn[s](t)
            elif s in csfn:
                csfn[s](t, vals.pop((s, t)))
            else:
                sfn[s](t)

        NS = 11
        # route mid-degree chunks to gpsimd until POOL_COLS columns
        engs = []
        pc = 0
        for (d, p0, np_, off) in chunks:
            cols = (d - 1) * np_ * P
            if POOL_DRANGE[0] <= d <= POOL_DRANGE[1] and pc < POOL_COLS:
                engs.append(nc.gpsimd)
                pc += cols
            else:
                engs.append(nc.vector)
        ci = 0
        while ci < len(chunks) and chunks[ci][1] < 6:
            emit_chunk(*chunks[ci], engs[ci])
            ci += 1
        for q in range(MT + NS):
            for s in range(NS):
                run_stage(s, q - s)
            need_p = 2 * (q + 4)
            while ci < len(chunks) and chunks[ci][1] < need_p:
                emit_chunk(*chunks[ci], engs[ci])
                ci += 1

    nc.compile()
    return nc


def _host_plan(col):
    """Degree-sort nodes, deal across cores, order buckets desc, pair."""
    NPAD = NCORES * NPC
    deg = np.zeros(NPAD, np.int64)
    deg[:N_NODES] = np.bincount(col, minlength=N_NODES)
    order = np.argsort(deg, kind="stable")          # ascending degree
    dsort = deg[order]
    d_blk = dsort.reshape(NB, NCORES * P).max(axis=1)
    d_blk = np.maximum(d_blk, 1).astype(np.int64)
    bo = np.argsort(-d_blk, kind="stable")          # desc degree
    d_new = d_blk[bo]
    dp0 = np.maximum(d_new[0::2], d_new[1::2])
    # start with 4 mid-degree pairs so the MLP pipeline fills immediately,
    # then heaviest-to-lightest (light pairs drain the tail fast)
    mid = NPAIR // 2
    porder = np.concatenate([np.arange(mid, mid + 4),
                             np.arange(0, mid),
                             np.arange(mid + 4, NPAIR)])
    d_pair = dp0[porder]
    bo = bo.reshape(NPAIR, 2)[porder].reshape(-1)
    inv_bo = np.empty(NB, np.int64)
    inv_bo[bo] = np.arange(NB)
    prefp = np.zeros(NPAIR, np.int64)
    np.cumsum(d_pair[:-1], out=prefp[1:])
    chunks = []
    p = 0
    while p < NPAIR:
        d = int(d_pair[p])
        e = p
        while e < NPAIR and d_pair[e] == d:
            e += 1
        cap = max(1, CHCAP // d)
        while p < e:
            np_ = min(cap, e - p)
            chunks.append((d, p, np_, int(prefp[p])))
            p += np_
    return order, dsort, bo, inv_bo, d_pair, prefp, tuple(chunks)


def _host_pack(col, edge_attr, order, dsort, inv_bo, prefp, Fp):
    E = col.shape[0]
    NPAD = NCORES * NPC
    pos = np.empty(NPAD, np.int64)
    pos[order] = np.arange(NPAD)
    pe = pos[col]                                    # sorted-pos of each dest
    eorder = np.argsort(pe, kind="stable")
    ps = pe[eorder]
    starts = np.zeros(NPAD, np.int64)
    np.cumsum(dsort[:-1], out=starts[1:])
    j = np.arange(E, dtype=np.int64) - starts[ps]
    c = ps % NCORES
    r = ps // NCORES
    b_old = r // P
    lane = r % P
    k = inv_bo[b_old]                                # new bucket index
    pair = k // 2
    half = k % 2
    COLS = Fp * P
    colp = (prefp[pair] + j) * P + lane
    rows = c * COLS + colp
    A = np.zeros((NCORES * COLS, P), BF16)
    av = np.asarray(edge_attr, np.float32)[eorder].astype(BF16)
    m0 = half == 0
    A[rows[m0], 0:D] = av[m0]
    A[rows[~m0], D:P] = av[~m0]
    return np.ascontiguousarray(
        A.reshape(NCORES, COLS, P).transpose(0, 2, 1))


def _host_x(x, ln_b, order, bo):
    NPAD = NCORES * NPC
    xpad = np.zeros((NPAD, D), np.float32)
    xpad[:N_NODES] = np.asarray(x, np.float32)
    # node at (core c, new bucket k, lane) = order[(bo[k]*128+lane)*8 + c]
    r_old = (bo[:, None] * P + np.arange(P)[None, :]).reshape(-1)   # [NPC]
    idx = order[r_old[:, None] * NCORES + np.arange(NCORES)[None, :]]
    xTs, xbs = [], []
    bln = np.asarray(ln_b, np.float32)[None, :]
    for cc in range(NCORES):
        xp = xpad[idx[:, cc]]                        # [NPC, 64]
        xTs.append(np.ascontiguousarray(xp.T).astype(BF16))
        xb = (xp + bln).reshape(NB, P, D).transpose(1, 0, 2).reshape(P, NB * D)
        xbs.append(np.ascontiguousarray(xb).astype(BF16))
    return idx, xTs, xbs


def _host_consts(W0, b0, W1, b1, W2, b2, ln_g):
    wb = np.zeros((P, 904), np.float32)
    W0 = np.asarray(W0, np.float32)
    wb[0:D, 0:H] = W0[0:D]                           # w0x
    wb[0:D, H:2 * H] = W0[D:2 * D]                   # [W0a; 0]
    wb[D:P, 2 * H:3 * H] = W0[D:2 * D]               # [0; W0a]
    wb[:, 384:512] = np.asarray(W1, np.float32)
    wb[:, 512:576] = np.asarray(W2, np.float32)
    wb[0:D, 576:640] = np.eye(D, dtype=np.float32)
    wb[:, 640:896] = np.broadcast_to(
        np.tile(np.asarray(ln_g, np.float32), 4), (P, 256))
    cf = np.zeros((P, 4), np.float32)
    cf[:, 0] = np.asarray(b0, np.float32)
    cf[:, 1] = np.asarray(b1, np.float32)
    cf[0:D, 2] = np.asarray(b2, np.float32)
    cf[:, 3] = EPS
    return wb.astype(BF16), cf


def kernel(x, edge_index, edge_attr, W0, b0, W1, b1, W2, b2, ln_g, ln_b):
    global last_results
    col = np.asarray(edge_index[1]).astype(np.int64)
    order, dsort, bo, inv_bo, d_pair, prefp, chunks = _host_plan(col)
    Fp = int(prefp[-1] + d_pair[-1])

    if chunks not in _prog_cache:
        _prog_cache[chunks] = _build_program(chunks)
    nc = _prog_cache[chunks]

    A = _host_pack(col, edge_attr, order, dsort, inv_bo, prefp, Fp)
    idx, xTs, xbs = _host_x(x, ln_b, order, bo)
    wb, cf = _host_consts(W0, b0, W1, b1, W2, b2, ln_g)

    in_maps = []
    for c in range(NCORES):
        in_maps.append({"attr": A[c], "xT": xTs[c], "xb": xbs[c],
                        "wb": wb, "cf": cf})

    res = run_bass_kernel_spmd(nc, in_maps, core_ids=list(range(NCORES)))
    last_results = res

    out = np.zeros((NCORES * NPC, D), np.float32)
    for c in range(NCORES):
        osw = res.results[c]["out"].astype(np.float32)   # [128, NB*64]
        o3 = osw.reshape(P, NB, D).transpose(1, 0, 2).reshape(NPC, D)
        out[idx[:, c]] = o3
    return np.ascontiguousarray(out[:N_NODES])
